# revision 1
# baseline (speedup 1.0000x reference)
"""GCNConv forward on 8 Trainium2 NeuronCores (Bass/Tile).

Strategy (graph/edge-cut parallelism):
  - Nodes are split into 784 buckets of 128 (98 buckets per core); each core
    owns the scatter-sum for its node shard.
  - deg/norm: each core counts out-degrees for its own nodes via one-hot
    (is_equal) tiles + a ones-matmul into PSUM.
  - g = norm[src] * x is computed distributed (own rows only) and shared with
    an AllGather (bf16), so per-edge messages are plain row-gathers of g.
  - Edges are grouped by destination bucket (host-side, data movement only);
    each 128-edge tile gathers its g[src] rows with one indirect DMA and
    scatter-adds them into the bucket's PSUM via a one-hot matmul.
  - Self-loops are one contiguous tile per bucket (identity matmul, no
    gather descriptors needed).
  - norm[dst] scaling cancels inside the final L2 normalization (deg >= 1
    because of self-loops), so it is skipped entirely.
  - out = tanh(L2-normalize(agg @ W)) with the L2/rsqrt done via Ln/Exp on
    the scalar engine (Rsqrt activation is banned for accuracy).
"""

import numpy as np
import ml_dtypes

N, E, D = 100000, 625000, 128
P = 128
NCORES = 8
NBUK = 784          # total dst buckets of 128 nodes
BPC = NBUK // NCORES  # 98 buckets per core
NPAD = NBUK * P     # 100352 padded node count
SHARD = BPC * P     # 12544 nodes per core
CHB = 7             # buckets per load/store group (98 = 14 * 7)
NGRP = BPC // CHB   # 14

_CACHE = {}


def _prep(edge_index):
    """Host-side partitioning: group edges by dst bucket (main stream) and by
    src bucket (degree-count stream). Pure data movement / index bookkeeping.
    Returns per-core device arrays + per-bucket-index tile capacities."""
    src = edge_index[0].astype(np.int64)
    dst = edge_index[1].astype(np.int64)

    def build_stream(bucket_of, slot_val, payload):
        # bucket_of: [E] bucket id per edge; slot_val: [E] 0..127 slot within
        # bucket; payload: [E] value to gather later (or None).
        order = np.argsort(bucket_of, kind="stable")
        b_sorted = bucket_of[order]
        counts = np.bincount(bucket_of, minlength=NBUK)
        starts = np.zeros(NBUK + 1, np.int64)
        np.cumsum(counts, out=starts[1:])
        pos = np.arange(len(order)) - starts[b_sorted]
        # per-bucket-INDEX capacity: max over the 8 cores that share a bl
        caps = np.ceil(counts.reshape(NCORES, BPC).max(0) / P).astype(np.int64)
        cum = np.zeros(BPC + 1, np.int64)
        np.cumsum(caps, out=cum[1:])
        tot = int(cum[-1])
        # flat slot index within a core's stream
        core = b_sorted // BPC
        bl = b_sorted % BPC
        t = pos // P
        prt = pos % P
        col = cum[bl] + t
        vals = np.zeros((NCORES, P, tot), np.int32)
        slots = np.full((NCORES, P, tot), 999.0, np.float32)
        slots_ok = slot_val[order]
        if payload is not None:
            vals[core, prt, col] = payload[order]
        slots[core, prt, col] = slots_ok
        return vals, slots, caps, cum, tot

    # main stream: real edges grouped by dst bucket; payload = g row (= src)
    e_src, e_dst, capE, cumE, totE = build_stream(dst // P, dst % P, src)
    # count stream: real edges grouped by src bucket; slot = src % P
    _, c_src, capC, cumC, totC = build_stream(src // P, src % P, None)
    return dict(
        e_src=e_src, e_dst=e_dst, capE=capE, cumE=cumE, totE=totE,
        c_src=c_src, capC=capC, cumC=cumC, totC=totC,
    )


def _build(capE, cumE, totE, capC, cumC, totC):
    import concourse.bass as bass
    import concourse.bacc as bacc
    import concourse.mybir as mybir
    import concourse.tile as tile

    F32 = mybir.dt.float32
    BF16 = mybir.dt.bfloat16
    I32 = mybir.dt.int32
    AF = mybir.ActivationFunctionType
    OP = mybir.AluOpType

    nc = bacc.Bacc("TRN2", target_bir_lowering=False, debug=False)
    x_sh = nc.dram_tensor("x_sh", [SHARD, D], F32, kind="ExternalInput")
    w_in = nc.dram_tensor("w_in", [D, D], F32, kind="ExternalInput")
    iota_in = nc.dram_tensor("iota_in", [P, P], BF16, kind="ExternalInput")
    iotac_in = nc.dram_tensor("iotac_in", [P, 1], F32, kind="ExternalInput")
    esrc_in = nc.dram_tensor("esrc_in", [P, totE], I32, kind="ExternalInput")
    edst_in = nc.dram_tensor("edst_in", [P, totE], F32, kind="ExternalInput")
    csrc_in = nc.dram_tensor("csrc_in", [P, totC], F32, kind="ExternalInput")
    out = nc.dram_tensor("out", [SHARD, D], F32, kind="ExternalOutput")

    with tile.TileContext(nc) as tc:
        with (
            tc.tile_pool(name="const", bufs=1) as cst,
            tc.tile_pool(name="inp", bufs=1) as inp,
            tc.tile_pool(name="spool", bufs=6) as spool,
            tc.tile_pool(name="xgpool", bufs=32) as xgp,
            tc.tile_pool(name="gx", bufs=2) as gxp,
            tc.tile_pool(name="gch", bufs=2) as gchp,
            tc.tile_pool(name="gself", bufs=2) as gsfp,
            tc.tile_pool(name="atp", bufs=3) as atp,
            tc.tile_pool(name="sqp", bufs=2) as sqp,
            tc.tile_pool(name="stage", bufs=1) as stg,
            tc.tile_pool(name="pcnt", bufs=2, space="PSUM") as pc,
            tc.tile_pool(name="pagg", bufs=2, space="PSUM") as pa,
            tc.tile_pool(name="pw", bufs=2, space="PSUM") as pw,
            tc.tile_pool(name="dram", bufs=1, space="DRAM") as drm,
        ):
            # ---- constants ----
            iota_t = cst.tile([P, P], BF16)
            iotac_t = cst.tile([P, 1], F32)
            w_sb = cst.tile([P, P], F32)
            w_bf = cst.tile([P, P], BF16)
            ident = cst.tile([P, P], BF16)
            ones_bf = cst.tile([P, 1], BF16)
            eps_t = cst.tile([P, 1], F32)
            nc.sync.dma_start(out=iota_t[:], in_=iota_in[:])
            nc.sync.dma_start(out=iotac_t[:], in_=iotac_in[:])
            nc.sync.dma_start(out=w_sb[:], in_=w_in[:])
            nc.vector.tensor_copy(w_bf[:], w_sb[:])
            nc.vector.tensor_scalar(
                out=ident[:], in0=iota_t[:], scalar1=iotac_t[:], scalar2=None,
                op0=OP.is_equal,
            )
            nc.gpsimd.memset(ones_bf[:], 1.0)
            nc.gpsimd.memset(eps_t[:], 1e-30)

            # ---- input streams ----
            esrc_t = inp.tile([P, totE], I32)
            edst_t = inp.tile([P, totE], F32)
            csrc_t = inp.tile([P, totC], F32)
            nc.sync.dma_start(out=esrc_t[:], in_=esrc_in[:])
            nc.sync.dma_start(out=edst_t[:], in_=edst_in[:])
            nc.sync.dma_start(out=csrc_t[:], in_=csrc_in[:])

            # ---- staging ----
            cnt_acc = stg.tile([P, BPC], F32)
            norm_own = stg.tile([P, BPC], F32)
            out_stage = stg.tile([P, BPC * P], F32)
            ssq = stg.tile([P, BPC], F32)
            rl2 = stg.tile([P, BPC], F32)

            g_own = drm.tile([SHARD, D], BF16)
            g_full = drm.tile([NPAD, D], BF16)

            # ---- phase A: out-degree counts for own nodes ----
            for bl in range(BPC):
                ncnt = int(capC[bl])
                ccol = pc.tile([P, 1], F32, space="PSUM")
                for t in range(ncnt):
                    col = int(cumC[bl]) + t
                    sC = spool.tile([P, P], BF16, tag="s")
                    nc.vector.tensor_scalar(
                        out=sC[:], in0=iota_t[:],
                        scalar1=csrc_t[:, col:col + 1], scalar2=None,
                        op0=OP.is_equal,
                    )
                    nc.tensor.matmul(
                        ccol[:], lhsT=sC[:], rhs=ones_bf[:],
                        start=(t == 0), stop=(t == ncnt - 1),
                    )
                nc.scalar.copy(out=cnt_acc[:, bl:bl + 1], in_=ccol[:])

            # norm = (deg+1)^-0.5 = exp(-0.5*ln(deg+1))
            nc.scalar.activation(norm_own[:], cnt_acc[:], AF.Ln, bias=1.0)
            nc.scalar.activation(norm_own[:], norm_own[:], AF.Exp, scale=-0.5)

            # ---- phase B: g_own = norm * x, then AllGather ----
            x_r = x_sh[:].rearrange("(b p) f -> p b f", p=P)
            gown_r = g_own[:].rearrange("(b p) f -> p b f", p=P)
            for grp in range(NGRP):
                sl = slice(grp * CHB, (grp + 1) * CHB)
                xch = gxp.tile([P, CHB, P], F32, tag="xch")
                nc.sync.dma_start(out=xch[:], in_=x_r[:, sl, :])
                gch = gchp.tile([P, CHB, P], BF16, tag="gch")
                for j in range(CHB):
                    bl = grp * CHB + j
                    nc.vector.tensor_scalar(
                        out=gch[:, j, :], in0=xch[:, j, :],
                        scalar1=norm_own[:, bl:bl + 1], scalar2=None,
                        op0=OP.mult,
                    )
                nc.sync.dma_start(out=gown_r[:, sl, :], in_=gch[:])
            nc.gpsimd.collective_compute(
                "AllGather",
                mybir.AluOpType.bypass,
                ins=[g_own.opt()],
                outs=[g_full.opt()],
                replica_groups=[list(range(NCORES))],
            )

            # ---- phase C: scatter-sum + W + L2-normalize + tanh ----
            for grp in range(NGRP):
                sl = slice(grp * CHB, (grp + 1) * CHB)
                gself = gsfp.tile([P, CHB, P], BF16, tag="gself")
                nc.sync.dma_start(out=gself[:], in_=gown_r[:, sl, :])
                for j in range(CHB):
                    bl = grp * CHB + j
                    ne = int(capE[bl])
                    pA = pa.tile([P, P], F32, space="PSUM")
                    # self-loop tile: A_T += g_self^T (identity one-hot)
                    nc.tensor.matmul(
                        pA[:], lhsT=gself[:, j, :], rhs=ident[:],
                        start=True, stop=(ne == 0),
                    )
                    for t in range(ne):
                        col = int(cumE[bl]) + t
                        xg = xgp.tile([P, P], BF16, tag="xg")
                        nc.gpsimd.indirect_dma_start(
                            out=xg[:], out_offset=None, in_=g_full[:],
                            in_offset=bass.IndirectOffsetOnAxis(
                                ap=esrc_t[:, col:col + 1], axis=0,
                            ),
                        )
                        sS = spool.tile([P, P], BF16, tag="s")
                        nc.vector.tensor_scalar(
                            out=sS[:], in0=iota_t[:],
                            scalar1=edst_t[:, col:col + 1], scalar2=None,
                            op0=OP.is_equal,
                        )
                        nc.tensor.matmul(
                            pA[:], lhsT=xg[:], rhs=sS[:],
                            start=False, stop=(t == ne - 1),
                        )
                    at = atp.tile([P, P], BF16, tag="at")
                    nc.scalar.copy(out=at[:], in_=pA[:])
                    pC = pw.tile([P, P], F32, space="PSUM")
                    nc.tensor.matmul(
                        pC[:], lhsT=at[:], rhs=w_bf[:], start=True, stop=True,
                    )
                    sq = sqp.tile([P, P], BF16, tag="sq")
                    nc.scalar.activation(
                        sq[:], pC[:], AF.Square, accum_out=ssq[:, bl:bl + 1],
                    )
                    nc.vector.tensor_copy(
                        out=out_stage[:, bl * P:(bl + 1) * P], in_=pC[:],
                    )

            # rl2 = 1/sqrt(ssq + eps); out = tanh(C * rl2)
            nc.scalar.activation(rl2[:], ssq[:], AF.Ln, bias=eps_t[:])
            nc.scalar.activation(rl2[:], rl2[:], AF.Exp, scale=-0.5)
            out_r = out[:].rearrange("(b p) f -> p b f", p=P)
            for grp in range(NGRP):
                for j in range(CHB):
                    bl = grp * CHB + j
                    nc.scalar.activation(
                        out_stage[:, bl * P:(bl + 1) * P],
                        out_stage[:, bl * P:(bl + 1) * P],
                        AF.Tanh, scale=rl2[:, bl:bl + 1],
                    )
                st3 = out_stage[:, grp * CHB * P:(grp + 1) * CHB * P]
                nc.sync.dma_start(
                    out=out_r[:, grp * CHB:(grp + 1) * CHB, :],
                    in_=st3.rearrange("p (b f) -> p b f", f=P),
                )

    nc.compile()
    return nc


def _make_in_maps(x, W, prep):
    iota_row = np.tile(
        np.arange(P, dtype=np.float32), (P, 1)
    ).astype(ml_dtypes.bfloat16)
    iota_col = np.arange(P, dtype=np.float32).reshape(P, 1)
    x_pad = np.zeros((NPAD, D), np.float32)
    x_pad[:N] = np.asarray(x, np.float32)
    w_np = np.asarray(W, np.float32)
    in_maps = []
    for c in range(NCORES):
        in_maps.append({
            "x_sh": np.ascontiguousarray(x_pad[c * SHARD:(c + 1) * SHARD]),
            "w_in": w_np,
            "iota_in": iota_row,
            "iotac_in": iota_col,
            "esrc_in": np.ascontiguousarray(prep["e_src"][c]),
            "edst_in": np.ascontiguousarray(prep["e_dst"][c]),
            "csrc_in": np.ascontiguousarray(prep["c_src"][c]),
        })
    return in_maps


def get_compiled(edge_index):
    """Build (or fetch cached) compiled program for this edge structure."""
    prep = _prep(np.asarray(edge_index))
    key = (tuple(prep["capE"]), tuple(prep["capC"]))
    if key not in _CACHE:
        _CACHE[key] = _build(
            prep["capE"], prep["cumE"], prep["totE"],
            prep["capC"], prep["cumC"], prep["totC"],
        )
    return _CACHE[key], prep


def kernel(x, edge_index, W):
    from concourse.bass_utils import run_bass_kernel_spmd

    nc, prep = get_compiled(edge_index)
    in_maps = _make_in_maps(x, W, prep)
    res = run_bass_kernel_spmd(nc, in_maps, core_ids=list(range(NCORES)))
    big = np.concatenate([res.results[c]["out"] for c in range(NCORES)], axis=0)
    return np.ascontiguousarray(big[:N]).astype(np.float32)
